# revision 35
# baseline (speedup 1.0000x reference)
"""Trainium2 multi-head attention kernel (8 NeuronCores).

Sharding: 2 (batch) x 4 (head-group) grid. Core c handles batch b=c//4 and
heads [4g, 4g+4) where g=c%4 (d_model slice of 256).

All-fp16 datapath (PSUM accumulates fp32), fully software-pipelined around
the ScalarE exp stream (~1016ns per [128,1024] tile), which is the binding
engine floor together with the PE:

  head (~15us): weights + first 1MB x chunks land, Q/K token-chunk 0
  projected, V token-group 0 projected, a dummy exp preloads the ACT table.
  phase C: 8 exp-paced iterations of scores (fp16 row-packed pairs) ->
  exp -> AV (lag 3). The remaining K chunks, V token-groups, and Q chunks
  are injected into early iterations' PE slack through the dD PSUM ring
  (epilogues on DVE), while their input DMAs stream under C. The output
  projection of each finished token chunk is interleaved at kt 6/9/12/15.

PSUM (8 banks): sc [128,1024] x2 (4) + av0/av1 [128,512] x1 (2) +
dD [128,512] x2 (2). Normalize copies av out through one [65,512] DVE copy
so the AV bank frees ~0.7us after the last AV matmul (enables av bufs=1).
"""
import sys

import numpy as np

for _p in ("/opt/trn_rl_repo", "/root/.axon_site/_ro/trn_rl_repo"):
    if _p not in sys.path:
        sys.path.append(_p)

import concourse.bacc as bacc
import concourse.mybir as mybir
import concourse.tile as tile
from concourse.bass_utils import run_bass_kernel_spmd

F32 = mybir.dt.float32
F16 = mybir.dt.float16

B, S, D, H, DK = 2, 2048, 1024, 16, 64
NC_ = 8
HG = D // 4          # 256: d_model slice per core
KT_D = D // 128      # 8 contraction tiles for projections
KT_S = S // 128      # 16 sequence tiles
QC = S // 512        # 4 query chunks of 512
LAG = 4              # AV lags exp by 4 kt
AF = mybir.ActivationFunctionType


def build_nc():
    nc = bacc.Bacc("TRN2", target_bir_lowering=False, debug=False, num_devices=NC_)

    # x tensors token-chunk swizzled: row cc*128+p, col kt*512+c
    xqT = nc.dram_tensor("xqT", [4 * 128, KT_D * 512], F16, kind="ExternalInput").ap()
    xkT = nc.dram_tensor("xkT", [4 * 128, KT_D * 512], F16, kind="ExternalInput").ap()
    xvT = nc.dram_tensor("xvT", [4 * 128, KT_D * 512], F16, kind="ExternalInput").ap()
    wqkT = nc.dram_tensor("wqkT", [128, 2 * KT_D * HG], F16,
                          kind="ExternalInput").ap()
    wvoT = nc.dram_tensor("wvoT", [128, 2 * KT_D * HG], F16,
                          kind="ExternalInput").ap()
    biasT = nc.dram_tensor("biasT", [128, 4 + HG + KT_D], F32,
                           kind="ExternalInput").ap()
    outT = nc.dram_tensor("outT", [128, QC * KT_D * 512], F16,
                          kind="ExternalOutput").ap()

    wqk_re = wqkT.rearrange("p (u kt c) -> p u kt c", u=2, kt=KT_D)
    out_re = outT.rearrange("p (qc ot c) -> p qc ot c", qc=QC, ot=KT_D)

    def xchunk(x, cc):
        return x[cc * 128:(cc + 1) * 128, :].rearrange(
            "p (kt c) -> p kt c", kt=KT_D)

    with tile.TileContext(nc) as tc:
        with (
            tc.tile_pool(name="const", bufs=1) as cpool,
            tc.tile_pool(name="proj", bufs=1) as ppool,
            tc.tile_pool(name="exp", bufs=8) as epool,
            tc.tile_pool(name="nrm", bufs=1) as npool,
            tc.tile_pool(name="ost", bufs=2) as opool,
            tc.tile_pool(name="psC", bufs=1, space="PSUM") as psC,
        ):
            # ---- persistent tiles ----
            w_qk = cpool.tile([128, 2, KT_D, HG], F16)
            wq_t, wk_t = w_qk[:, 0], w_qk[:, 1]
            w_vo = cpool.tile([128, 2 * KT_D * HG], F16)
            wv_t = w_vo[:, 0:2048].rearrange("p (kt c) -> p kt c", kt=KT_D)
            wo_t = w_vo[:, 2048:4096].rearrange("p (k2 c) -> p k2 c", k2=2)
            bias_t = cpool.tile([128, 4 + HG + KT_D], F32)
            bqv_t = bias_t[:, 0:4]
            bvb_t = bias_t[:, 4:4 + HG]
            bob_t = bias_t[:, 4 + HG:4 + HG + KT_D]
            dmy = cpool.tile([1, 2], F32)
            xq_all = cpool.tile([128, QC, KT_D, 512], F16)
            xk_all = cpool.tile([128, QC, KT_D, 512], F16)
            xv_all = cpool.tile([128, QC, KT_D, 512], F16)

            qT = ppool.tile([128, 2, S], F16)  # [o-part, Mtile, t]
            kT = ppool.tile([128, 2, S], F16)
            vS = ppool.tile([128, KT_S, 4 * 128], F16)  # [t-part, t-tile, head*65]
            aoT = ppool.tile([128, 2, S], F16)

            # ---- DMA streams ----
            # HWDGE DMAs serialize per ring (~2us completion each), so x
            # chunks are split across BOTH rings in deadline order
            nc.sync.dma_start(xq_all[:, 0], xchunk(xqT, 0))
            nc.sync.dma_start(xk_all[:, 0], xchunk(xkT, 0))
            nc.sync.dma_start(xk_all[:, 1], xchunk(xkT, 1))
            nc.sync.dma_start(xk_all[:, 3], xchunk(xkT, 3))
            nc.sync.dma_start(xq_all[:, 1], xchunk(xqT, 1))
            nc.sync.dma_start(xq_all[:, 3], xchunk(xqT, 3))
            nc.scalar.dma_start(w_qk[:], wqk_re[:])
            nc.scalar.dma_start(w_vo[:, 0:2048], wvoT[:, 0:2048])
            nc.scalar.dma_start(bias_t[:], biasT[:])
            nc.scalar.dma_start(xv_all[:, 0], xchunk(xvT, 0))
            nc.scalar.dma_start(xv_all[:, 1], xchunk(xvT, 1))
            nc.scalar.dma_start(xk_all[:, 2], xchunk(xkT, 2))
            nc.scalar.dma_start(xv_all[:, 2], xchunk(xvT, 2))
            nc.scalar.dma_start(xv_all[:, 3], xchunk(xvT, 3))
            nc.scalar.dma_start(w_vo[:, 2048:4096], wvoT[:, 2048:4096])
            nc.scalar.dma_start(xq_all[:, 2], xchunk(xqT, 2))

            # preload the exp ACT table off the critical path
            nc.gpsimd.memset(dmy[:], 0.0)
            nc.scalar.activation(dmy[0:1, 1:2], dmy[0:1, 0:1], AF.Exp)
            nc.gpsimd.memset(vS[:], 1.0)

            # ---- projection chunk emitters ----
            def qk_chunk(w_t, dst, x_all, cc, bcol, on_vector):
                """Project one 512-token chunk of Q or K (both Mtiles)."""
                t = psC.tile([128, 1024], F32, name=f"pjc{cc}", tag="sc",
                             bufs=2)
                for kt in range(KT_D):
                    for m in range(2):
                        nc.tensor.matmul(
                            t[:, m * 512:(m + 1) * 512],
                            w_t[:, kt, m * 128:(m + 1) * 128],
                            x_all[:, cc, kt, :],
                            start=(kt == 0), stop=(kt == KT_D - 1))
                for m in range(2):
                    o = dst[:, m, cc * 512:(cc + 1) * 512]
                    i_ = t[:, m * 512:(m + 1) * 512]
                    b = bqv_t[:, bcol + m:bcol + m + 1]
                    if on_vector:
                        nc.vector.tensor_scalar_add(o, i_, b)
                    else:
                        nc.scalar.activation(o, i_, AF.Identity, bias=b)

            def qk_chunk_steps(w_t, dst, x_all, cc, bcol):
                """Split chunk into 4 pacing steps (4 MMs each) + DVE epi.

                The PSUM tile is allocated inside step 0 so its dD-ring
                position matches emission order (alloc at build time would
                deadlock against V chains emitted in between)."""
                hold = {}

                def step(s):
                    if s == 0:
                        hold["t"] = [
                            psC.tile([128, 512], F32, name=f"pji{cc}{m}",
                                     tag="dD", bufs=2)
                            for m in range(2)
                        ]
                    t = hold["t"]
                    for kt in range(4 * s, 4 * s + 4):
                        for m in range(2):
                            nc.tensor.matmul(
                                t[m][:],
                                w_t[:, kt, m * 128:(m + 1) * 128],
                                x_all[:, cc, kt, :],
                                start=(kt == 0), stop=(kt == KT_D - 1))
                    if s == 1:
                        for m in range(2):
                            nc.vector.tensor_scalar_add(
                                dst[:, m, cc * 512:(cc + 1) * 512],
                                t[m][:],
                                bqv_t[:, bcol + m:bcol + m + 1])
                return [lambda s=s: step(s) for s in range(2)]

            def v_tchain(tg, t):
                """One [128 tokens] V projection chain + vS writes."""
                ps = psC.tile([128, HG], F32, name=f"psv{tg}{t}",
                              tag=("av0", "av1", "dD", "dD")[t] if tg == 0
                              else "dD",
                              bufs=(1, 1, 2, 2)[t] if tg == 0 else 2)
                for kt in range(KT_D):
                    nc.tensor.matmul(
                        ps[:],
                        xv_all[:, tg, kt, t * 128:(t + 1) * 128],
                        wv_t[:, kt, :], start=(kt == 0),
                        stop=(kt == KT_D - 1))
                tt = tg * 4 + t
                for h in range(4):
                    nc.vector.tensor_tensor(
                        vS[:, tt, h * 128:h * 128 + 64],
                        ps[:, h * 64:(h + 1) * 64],
                        bvb_t[:, h * 64:(h + 1) * 64],
                        op=mybir.AluOpType.add)

            # ---- head: Q/K chunk 0 + V token-group 0 ----
            qk_chunk(wq_t, qT, xq_all, 0, 0, on_vector=False)
            qk_chunk(wk_t, kT, xk_all, 0, 2, on_vector=False)
            for t in range(4):
                v_tchain(0, t)

            # ---- injection schedule: (qcp, kt) -> emitters ----
            # dD-ring users must be emitted in strict sequential order
            # (an alloc may only wait on releases of earlier-emitted work).
            from collections import defaultdict
            inj = defaultdict(list)

            def add_qk(w_t, dst, x_all, cc, bcol, qcp0, kt0):
                steps = qk_chunk_steps(w_t, dst, x_all, cc, bcol)
                for s in range(2):
                    inj[(qcp0, kt0 + s)].append(steps[s])

            add_qk(wk_t, kT, xk_all, 1, 2, 0, 0)          # K-c1 kt0-1
            for t in range(4):                             # V-tg1 kt2-5
                inj[(0, 2 + t)].append(lambda t=t: v_tchain(1, t))
            add_qk(wk_t, kT, xk_all, 2, 2, 0, 6)          # K-c2 kt6-7
            add_qk(wk_t, kT, xk_all, 3, 2, 0, 8)          # K-c3 kt8-9
            for t in range(4):                             # V-tg2 kt10-13
                inj[(0, 10 + t)].append(lambda t=t: v_tchain(2, t))
            for t in range(2):                             # V-tg3 kt14-15
                inj[(0, 14 + t)].append(lambda t=t: v_tchain(3, t))
            for t in range(2, 4):                          # V-tg3 rest
                inj[(1, t - 2)].append(lambda t=t: v_tchain(3, t))
            add_qk(wq_t, qT, xq_all, 1, 0, 1, 4)          # Q-c1 iter1 kt4-5
            add_qk(wq_t, qT, xq_all, 2, 0, 2, 4)          # Q-c2 iter2 kt4-5
            add_qk(wq_t, qT, xq_all, 3, 0, 3, 4)          # Q-c3 iter3 kt4-5

            # ---- phase C ----
            state = {}

            def av_mms(st, kt):
                for i in range(2):
                    nc.tensor.matmul(
                        st["av"][i][:],
                        vS[:, kt, (2 * st["p"] + i) * 128:
                           (2 * st["p"] + i + 1) * 128],
                        st["ex"][kt][:, i * 512:(i + 1) * 512],
                        start=(kt == 0), stop=(kt == KT_S - 1))

            def normalize(st):
                # head i's gpsimd broadcast overlaps head i+1's DVE chain
                p, qc = st["p"], st["qc"]
                uos, rbs = [], []
                for i in range(2):
                    uo = npool.tile([65, 512], F32, name="uo", tag=f"uo{i}")
                    nc.vector.tensor_copy(uo[:], st["av"][i][0:65, :])
                    uos.append(uo)
                for i in range(2):
                    sr = npool.tile([1, 512], F32, name="sr", tag=f"sr{i}")
                    nc.vector.tensor_copy(sr[:], uos[i][64:65, :])
                    rc = npool.tile([1, 512], F32, name="rc", tag=f"rc{i}")
                    scr = npool.tile([1, 512], F32, name="scr", tag=f"scr{i}")
                    nc.vector.reciprocal_approx_accurate(rc[:], sr[:], scr[:])
                    rb = npool.tile([64, 512], F32, name="rb", tag=f"rb{i}")
                    nc.gpsimd.partition_broadcast(rb[:], rc[:])
                    rbs.append(rb)
                for i in range(2):
                    nc.vector.tensor_tensor(
                        aoT[i * 64:(i + 1) * 64, p, qc * 512:(qc + 1) * 512],
                        uos[i][0:64, :], rbs[i][:], op=mybir.AluOpType.mult)

            def emit_D_pair(qc, ot0):
                for ot in (ot0, ot0 + 1):
                    acc2 = psC.tile([128, 512], F32, name=f"acc2{ot}",
                                    tag="dD", bufs=2)
                    for k2 in range(2):
                        nc.tensor.matmul(
                            acc2[:],
                            wo_t[:, k2, ot * 128:(ot + 1) * 128],
                            aoT[:, k2, qc * 512:(qc + 1) * 512],
                            start=(k2 == 0), stop=(k2 == 1))
                    nc.vector.tensor_scalar_add(
                        state["o_big"][:, ot, :], acc2[:], bob_t[:, ot:ot + 1])

            prev = None
            for qcp in range(2 * QC):
                qc, p = qcp // 2, qcp % 2
                cur = {"qc": qc, "p": p, "ex": [], "av": None}
                do_D = (p == 0 and qc > 0)
                for kt in range(KT_S):
                    sc = psC.tile([128, 1024], F32, name="sc", tag="sc",
                                  bufs=2)
                    nc.tensor.matmul(
                        sc[:, 0:512],
                        kT[0:64, p, kt * 128:(kt + 1) * 128],
                        qT[0:64, p, qc * 512:(qc + 1) * 512],
                        start=True, stop=True, tile_position=(0, 0))
                    nc.tensor.matmul(
                        sc[:, 512:1024],
                        kT[64:128, p, kt * 128:(kt + 1) * 128],
                        qT[64:128, p, qc * 512:(qc + 1) * 512],
                        start=True, stop=True, tile_position=(64, 0))
                    ex = epool.tile([128, 1024], F16, name="ex", tag="ex")
                    nc.scalar.activation(ex[:], sc[:], AF.Exp, scale=0.125)
                    cur["ex"].append(ex)

                    # previous iteration's AV tail + normalize, after this
                    # iteration's scores so the exp stream never gaps
                    if prev is not None and kt < LAG - 1:
                        av_mms(prev, KT_S - LAG + kt)
                    if kt == LAG - 1:
                        if prev is not None:
                            av_mms(prev, KT_S - 1)
                            normalize(prev)
                        cur["av"] = [
                            psC.tile([128, 512], F32, name=f"av{i}",
                                     tag=f"av{i}", bufs=1)
                            for i in range(2)
                        ]
                    for fn in inj.get((qcp, kt), ()):
                        fn()
                    if do_D:
                        if kt == 10:
                            state["o_big"] = opool.tile(
                                [128, KT_D, 512], F16, name="o_big",
                                tag="o_big")
                            emit_D_pair(qc - 1, 0)
                        elif kt == 11:
                            emit_D_pair(qc - 1, 2)
                        elif kt == 12:
                            emit_D_pair(qc - 1, 4)
                            nc.sync.dma_start(
                                out_re[:, qc - 1, 0:4, :],
                                state["o_big"][:, 0:4, :])
                        elif kt == 13:
                            emit_D_pair(qc - 1, 6)
                        elif kt == 15:
                            nc.scalar.dma_start(
                                out_re[:, qc - 1, 4:8, :],
                                state["o_big"][:, 4:8, :])
                    if kt >= LAG:
                        av_mms(cur, kt - LAG)
                prev = cur

            # tail
            for kt in range(KT_S - LAG, KT_S):
                av_mms(prev, kt)
            normalize(prev)
            state["o_big"] = opool.tile(
                [128, KT_D, 512], F16, name="o_big", tag="o_big")
            for ot0 in (0, 2):
                emit_D_pair(QC - 1, ot0)
            nc.sync.dma_start(
                out_re[:, QC - 1, 0:4, :], state["o_big"][:, 0:4, :])
            for ot0 in (4, 6):
                emit_D_pair(QC - 1, ot0)
            nc.scalar.dma_start(
                out_re[:, QC - 1, 4:8, :], state["o_big"][:, 4:8, :])

    nc.compile()
    return nc


_NC = None


def _get_nc():
    global _NC
    if _NC is None:
        _NC = build_nc()
    return _NC


def _swz_w(a, inner):
    """[inner*128, C] -> [128, inner*C] partition-major."""
    rows, C = a.shape
    return np.ascontiguousarray(
        a.reshape(inner, 128, C).transpose(1, 0, 2).reshape(128, inner * C))


def _swz_tok(a):
    """[1024, 2048] -> [512, 4096]: row cc*128+p, col kt*512+c."""
    return np.ascontiguousarray(
        a.reshape(8, 128, 4, 512).transpose(2, 1, 0, 3).reshape(512, 4096))


def kernel(q, k, v, Wq, bq, Wk, bk, Wv, bv, Wo, bo):
    nc = _get_nc()

    q = np.asarray(q, np.float32)
    k = np.asarray(k, np.float32)
    v = np.asarray(v, np.float32)

    xT = {}
    for b in range(B):
        xT[("q", b)] = _swz_tok(np.ascontiguousarray(q[b].T).astype(np.float16))
        xT[("k", b)] = _swz_tok(np.ascontiguousarray(k[b].T).astype(np.float16))
        xT[("v", b)] = _swz_tok(np.ascontiguousarray(v[b].T).astype(np.float16))

    WqT = np.asarray(Wq, np.float32).T.astype(np.float16)
    WkT = np.asarray(Wk, np.float32).T.astype(np.float16)
    WvT = np.asarray(Wv, np.float32).T.astype(np.float16)
    WoT = np.asarray(Wo, np.float32).T.astype(np.float16)
    bq = np.asarray(bq, np.float32)
    bk = np.asarray(bk, np.float32)
    bv = np.asarray(bv, np.float32)
    bo = np.asarray(bo, np.float32)

    in_maps = []
    for c in range(NC_):
        b, g = divmod(c, 4)
        sl = slice(g * HG, (g + 1) * HG)
        bqs, bks = bq[sl], bk[sl]
        bqv_a = np.stack(
            [bqs[0:128], bqs[128:256], bks[0:128], bks[128:256]], axis=1)
        bo_a = (bo if g == 0 else np.zeros_like(bo)).reshape(KT_D, 128).T
        wqk = np.concatenate(
            [_swz_w(np.ascontiguousarray(WqT[:, sl]), KT_D),
             _swz_w(np.ascontiguousarray(WkT[:, sl]), KT_D)], axis=1)
        wvo = np.concatenate(
            [_swz_w(np.ascontiguousarray(WvT[:, sl]), KT_D),
             _swz_w(np.ascontiguousarray(WoT[sl, :]), 2)], axis=1)
        bias_m = np.concatenate(
            [bqv_a, np.broadcast_to(bv[sl], (128, HG)), bo_a],
            axis=1).astype(np.float32)
        in_maps.append({
            "xqT": xT[("q", b)],
            "xkT": xT[("k", b)],
            "xvT": xT[("v", b)],
            "wqkT": np.ascontiguousarray(wqk),
            "wvoT": np.ascontiguousarray(wvo),
            "biasT": np.ascontiguousarray(bias_m),
        })

    res = run_bass_kernel_spmd(nc, in_maps, list(range(NC_)))

    out = np.empty((B, S, D), np.float32)
    for b in range(B):
        acc = np.zeros((128, QC, KT_D, 512), np.float32)
        for g in range(4):
            acc += res.results[b * 4 + g]["outT"].astype(np.float32).reshape(
                128, QC, KT_D, 512)
        oT = acc.transpose(2, 0, 1, 3).reshape(D, S)
        out[b] = oT.T
    return out


# revision 38
# speedup vs baseline: 1.0315x; 1.0315x over previous
"""Trainium2 multi-head attention kernel (8 NeuronCores).

Sharding: 2 (batch) x 4 (head-group) grid. Core c handles batch b=c//4 and
heads [4g, 4g+4) where g=c%4 (d_model slice of 256).

All-fp16 datapath (PSUM accumulates fp32), fully software-pipelined around
the ScalarE exp stream (~1016ns per [128,1024] tile), which is the binding
engine floor together with the PE:

  head (~15us): weights + first 1MB x chunks land, Q/K token-chunk 0
  projected, V token-group 0 projected, a dummy exp preloads the ACT table.
  phase C: 8 exp-paced iterations of scores (fp16 row-packed pairs) ->
  exp -> AV (lag 3). The remaining K chunks, V token-groups, and Q chunks
  are injected into early iterations' PE slack through the dD PSUM ring
  (epilogues on DVE), while their input DMAs stream under C. The output
  projection of each finished token chunk is interleaved at kt 6/9/12/15.

PSUM (8 banks): sc [128,1024] x2 (4) + av0/av1 [128,512] x1 (2) +
dD [128,512] x2 (2). Normalize copies av out through one [65,512] DVE copy
so the AV bank frees ~0.7us after the last AV matmul (enables av bufs=1).
"""
import sys

import numpy as np

for _p in ("/opt/trn_rl_repo", "/root/.axon_site/_ro/trn_rl_repo"):
    if _p not in sys.path:
        sys.path.append(_p)

import concourse.bacc as bacc
import concourse.mybir as mybir
import concourse.tile as tile
from concourse.bass_utils import run_bass_kernel_spmd

F32 = mybir.dt.float32
F16 = mybir.dt.float16

B, S, D, H, DK = 2, 2048, 1024, 16, 64
NC_ = 8
HG = D // 4          # 256: d_model slice per core
KT_D = D // 128      # 8 contraction tiles for projections
KT_S = S // 128      # 16 sequence tiles
QC = S // 512        # 4 query chunks of 512
LAG = 4              # AV lags exp by 4 kt
AF = mybir.ActivationFunctionType


def build_nc():
    nc = bacc.Bacc("TRN2", target_bir_lowering=False, debug=False, num_devices=NC_)

    # x tensors token-chunk swizzled: row cc*128+p, col kt*512+c
    xqT = nc.dram_tensor("xqT", [4 * 128, KT_D * 512], F16, kind="ExternalInput").ap()
    xkT = nc.dram_tensor("xkT", [4 * 128, KT_D * 512], F16, kind="ExternalInput").ap()
    xvT = nc.dram_tensor("xvT", [4 * 128, KT_D * 512], F16, kind="ExternalInput").ap()
    wqkT = nc.dram_tensor("wqkT", [128, 2 * KT_D * HG], F16,
                          kind="ExternalInput").ap()
    wvoT = nc.dram_tensor("wvoT", [128, 2 * KT_D * HG], F16,
                          kind="ExternalInput").ap()
    biasT = nc.dram_tensor("biasT", [128, 4 + HG + KT_D], F32,
                           kind="ExternalInput").ap()
    outT = nc.dram_tensor("outT", [128, QC * KT_D * 512], F16,
                          kind="ExternalOutput").ap()

    wqk_re = wqkT.rearrange("p (u kt c) -> p u kt c", u=2, kt=KT_D)
    out_re = outT.rearrange("p (qc ot c) -> p qc ot c", qc=QC, ot=KT_D)

    def xchunk(x, cc):
        return x[cc * 128:(cc + 1) * 128, :].rearrange(
            "p (kt c) -> p kt c", kt=KT_D)

    with tile.TileContext(nc) as tc:
        with (
            tc.tile_pool(name="const", bufs=1) as cpool,
            tc.tile_pool(name="proj", bufs=1) as ppool,
            tc.tile_pool(name="exp", bufs=8) as epool,
            tc.tile_pool(name="nrm", bufs=1) as npool,
            tc.tile_pool(name="ost", bufs=2) as opool,
            tc.tile_pool(name="psC", bufs=1, space="PSUM") as psC,
        ):
            # ---- persistent tiles ----
            w_qk = cpool.tile([128, 2, KT_D, HG], F16)
            wq_t, wk_t = w_qk[:, 0], w_qk[:, 1]
            w_vo = cpool.tile([128, 2 * KT_D * HG], F16)
            wv_t = w_vo[:, 0:2048].rearrange("p (kt c) -> p kt c", kt=KT_D)
            wo_t = w_vo[:, 2048:4096].rearrange("p (k2 c) -> p k2 c", k2=2)
            bias_t = cpool.tile([128, 4 + HG + KT_D], F32)
            bqv_t = bias_t[:, 0:4]
            bvb_t = bias_t[:, 4:4 + HG]
            bob_t = bias_t[:, 4 + HG:4 + HG + KT_D]
            dmy = cpool.tile([1, 2], F32)
            xq_all = cpool.tile([128, QC, KT_D, 512], F16)
            xk_all = cpool.tile([128, QC, KT_D, 512], F16)
            xv_all = cpool.tile([128, QC, KT_D, 512], F16)

            qT = ppool.tile([128, 2, S], F16)  # [o-part, Mtile, t]
            kT = ppool.tile([128, 2, S], F16)
            vS = ppool.tile([128, KT_S, 4 * 128], F16)  # [t-part, t-tile, head*65]
            aoT = ppool.tile([128, 2, S], F16)

            # ---- DMA streams ----
            # HWDGE DMAs serialize per ring (~2us completion each), so x
            # chunks are split across BOTH rings in deadline order
            nc.sync.dma_start(xq_all[:, 0], xchunk(xqT, 0))
            nc.sync.dma_start(xk_all[:, 0], xchunk(xkT, 0))
            nc.sync.dma_start(xv_all[:, 0], xchunk(xvT, 0))
            nc.sync.dma_start(xk_all[:, 1], xchunk(xkT, 1))
            nc.sync.dma_start(xk_all[:, 3], xchunk(xkT, 3))
            nc.sync.dma_start(xq_all[:, 1], xchunk(xqT, 1))
            nc.sync.dma_start(xq_all[:, 3], xchunk(xqT, 3))
            nc.scalar.dma_start(w_qk[:], wqk_re[:])
            nc.scalar.dma_start(bias_t[:], biasT[:])
            nc.scalar.dma_start(w_vo[:, 0:2048], wvoT[:, 0:2048])
            nc.scalar.dma_start(xv_all[:, 1], xchunk(xvT, 1))
            nc.scalar.dma_start(xk_all[:, 2], xchunk(xkT, 2))
            nc.scalar.dma_start(xv_all[:, 2], xchunk(xvT, 2))
            nc.scalar.dma_start(xv_all[:, 3], xchunk(xvT, 3))
            nc.scalar.dma_start(w_vo[:, 2048:4096], wvoT[:, 2048:4096])
            nc.scalar.dma_start(xq_all[:, 2], xchunk(xqT, 2))

            # preload the exp ACT table off the critical path
            nc.gpsimd.memset(dmy[:], 0.0)
            nc.scalar.activation(dmy[0:1, 1:2], dmy[0:1, 0:1], AF.Exp)
            nc.gpsimd.memset(vS[:], 1.0)

            # ---- projection chunk emitters ----
            def qk_chunk(w_t, dst, x_all, cc, bcol, on_vector):
                """Project one 512-token chunk of Q or K (both Mtiles)."""
                t = psC.tile([128, 1024], F32, name=f"pjc{cc}", tag="sc",
                             bufs=2)
                for kt in range(KT_D):
                    for m in range(2):
                        nc.tensor.matmul(
                            t[:, m * 512:(m + 1) * 512],
                            w_t[:, kt, m * 128:(m + 1) * 128],
                            x_all[:, cc, kt, :],
                            start=(kt == 0), stop=(kt == KT_D - 1))
                for m in range(2):
                    o = dst[:, m, cc * 512:(cc + 1) * 512]
                    i_ = t[:, m * 512:(m + 1) * 512]
                    b = bqv_t[:, bcol + m:bcol + m + 1]
                    if on_vector:
                        nc.vector.tensor_scalar_add(o, i_, b)
                    else:
                        nc.scalar.activation(o, i_, AF.Identity, bias=b)

            def qk_chunk_steps(w_t, dst, x_all, cc, bcol):
                """Split chunk into 4 pacing steps (4 MMs each) + DVE epi.

                The PSUM tile is allocated inside step 0 so its dD-ring
                position matches emission order (alloc at build time would
                deadlock against V chains emitted in between)."""
                hold = {}

                def step(s):
                    if s == 0:
                        hold["t"] = [
                            psC.tile([128, 512], F32, name=f"pji{cc}{m}",
                                     tag="dD", bufs=2)
                            for m in range(2)
                        ]
                    t = hold["t"]
                    for kt in range(4 * s, 4 * s + 4):
                        for m in range(2):
                            nc.tensor.matmul(
                                t[m][:],
                                w_t[:, kt, m * 128:(m + 1) * 128],
                                x_all[:, cc, kt, :],
                                start=(kt == 0), stop=(kt == KT_D - 1))
                    if s == 1:
                        for m in range(2):
                            nc.vector.tensor_scalar_add(
                                dst[:, m, cc * 512:(cc + 1) * 512],
                                t[m][:],
                                bqv_t[:, bcol + m:bcol + m + 1])
                return [lambda s=s: step(s) for s in range(2)]

            def v_tchain(tg, t):
                """One [128 tokens] V projection chain + vS writes."""
                ps = psC.tile([128, HG], F32, name=f"psv{tg}{t}",
                              tag=("av0", "av1", "dD", "dD")[t] if tg == 0
                              else "dD",
                              bufs=(1, 1, 2, 2)[t] if tg == 0 else 2)
                for kt in range(KT_D):
                    nc.tensor.matmul(
                        ps[:],
                        xv_all[:, tg, kt, t * 128:(t + 1) * 128],
                        wv_t[:, kt, :], start=(kt == 0),
                        stop=(kt == KT_D - 1))
                tt = tg * 4 + t
                for h in range(4):
                    nc.vector.tensor_tensor(
                        vS[:, tt, h * 128:h * 128 + 64],
                        ps[:, h * 64:(h + 1) * 64],
                        bvb_t[:, h * 64:(h + 1) * 64],
                        op=mybir.AluOpType.add)

            # ---- head: Q/K chunk 0 + V token-group 0 ----
            qk_chunk(wq_t, qT, xq_all, 0, 0, on_vector=False)
            qk_chunk(wk_t, kT, xk_all, 0, 2, on_vector=False)
            for t in range(4):
                v_tchain(0, t)

            # ---- injection schedule: (qcp, kt) -> emitters ----
            # dD-ring users must be emitted in strict sequential order
            # (an alloc may only wait on releases of earlier-emitted work).
            from collections import defaultdict
            inj = defaultdict(list)

            def add_qk(w_t, dst, x_all, cc, bcol, qcp0, kt0):
                steps = qk_chunk_steps(w_t, dst, x_all, cc, bcol)
                for s in range(2):
                    inj[(qcp0, kt0 + s)].append(steps[s])

            add_qk(wk_t, kT, xk_all, 1, 2, 0, 0)          # K-c1 kt0-1
            for t in range(4):                             # V-tg1 kt2-5
                inj[(0, 2 + t)].append(lambda t=t: v_tchain(1, t))
            add_qk(wk_t, kT, xk_all, 2, 2, 0, 6)          # K-c2 kt6-7
            add_qk(wk_t, kT, xk_all, 3, 2, 0, 8)          # K-c3 kt8-9
            for t in range(4):                             # V-tg2 kt10-13
                inj[(0, 10 + t)].append(lambda t=t: v_tchain(2, t))
            for t in range(2):                             # V-tg3 kt14-15
                inj[(0, 14 + t)].append(lambda t=t: v_tchain(3, t))
            for t in range(2, 4):                          # V-tg3 rest
                inj[(1, t - 2)].append(lambda t=t: v_tchain(3, t))
            add_qk(wq_t, qT, xq_all, 1, 0, 1, 4)          # Q-c1 iter1 kt4-5
            add_qk(wq_t, qT, xq_all, 2, 0, 2, 0)          # Q-c2 iter2 kt0-1
            add_qk(wq_t, qT, xq_all, 3, 0, 3, 0)          # Q-c3 iter3 kt0-1

            # ---- phase C ----
            state = {}

            def av_mms(st, kt):
                for i in range(2):
                    nc.tensor.matmul(
                        st["av"][i][:],
                        vS[:, kt, (2 * st["p"] + i) * 128:
                           (2 * st["p"] + i + 1) * 128],
                        st["ex"][kt][:, i * 512:(i + 1) * 512],
                        start=(kt == 0), stop=(kt == KT_S - 1))

            def normalize(st):
                # head i's gpsimd broadcast overlaps head i+1's DVE chain
                p, qc = st["p"], st["qc"]
                uos, rbs = [], []
                for i in range(2):
                    uo = npool.tile([65, 512], F32, name="uo", tag=f"uo{i}")
                    nc.vector.tensor_copy(uo[:], st["av"][i][0:65, :])
                    uos.append(uo)
                for i in range(2):
                    sr = npool.tile([1, 512], F32, name="sr", tag=f"sr{i}")
                    nc.vector.tensor_copy(sr[:], uos[i][64:65, :])
                    rc = npool.tile([1, 512], F32, name="rc", tag=f"rc{i}")
                    scr = npool.tile([1, 512], F32, name="scr", tag=f"scr{i}")
                    nc.vector.reciprocal_approx_accurate(rc[:], sr[:], scr[:])
                    rb = npool.tile([64, 512], F32, name="rb", tag=f"rb{i}")
                    nc.gpsimd.partition_broadcast(rb[:], rc[:])
                    rbs.append(rb)
                for i in range(2):
                    nc.vector.tensor_tensor(
                        aoT[i * 64:(i + 1) * 64, p, qc * 512:(qc + 1) * 512],
                        uos[i][0:64, :], rbs[i][:], op=mybir.AluOpType.mult)

            def emit_D_pair(qc, ot0):
                for ot in (ot0, ot0 + 1):
                    acc2 = psC.tile([128, 512], F32, name=f"acc2{ot}",
                                    tag="dD", bufs=2)
                    for k2 in range(2):
                        nc.tensor.matmul(
                            acc2[:],
                            wo_t[:, k2, ot * 128:(ot + 1) * 128],
                            aoT[:, k2, qc * 512:(qc + 1) * 512],
                            start=(k2 == 0), stop=(k2 == 1))
                    nc.vector.tensor_scalar_add(
                        state["o_big"][:, ot, :], acc2[:], bob_t[:, ot:ot + 1])

            prev = None
            for qcp in range(2 * QC):
                qc, p = qcp // 2, qcp % 2
                cur = {"qc": qc, "p": p, "ex": [], "av": None}
                do_D = (p == 0 and qc > 0)
                for kt in range(KT_S):
                    sc = psC.tile([128, 1024], F32, name="sc", tag="sc",
                                  bufs=2)
                    nc.tensor.matmul(
                        sc[:, 0:512],
                        kT[0:64, p, kt * 128:(kt + 1) * 128],
                        qT[0:64, p, qc * 512:(qc + 1) * 512],
                        start=True, stop=True, tile_position=(0, 0))
                    nc.tensor.matmul(
                        sc[:, 512:1024],
                        kT[64:128, p, kt * 128:(kt + 1) * 128],
                        qT[64:128, p, qc * 512:(qc + 1) * 512],
                        start=True, stop=True, tile_position=(64, 0))
                    ex = epool.tile([128, 1024], F16, name="ex", tag="ex")
                    nc.scalar.activation(ex[:], sc[:], AF.Exp, scale=0.125)
                    cur["ex"].append(ex)

                    # previous iteration's AV tail + normalize, after this
                    # iteration's scores so the exp stream never gaps
                    if prev is not None and kt < LAG - 1:
                        av_mms(prev, KT_S - LAG + kt)
                    if kt == LAG - 1:
                        if prev is not None:
                            av_mms(prev, KT_S - 1)
                            normalize(prev)
                        cur["av"] = [
                            psC.tile([128, 512], F32, name=f"av{i}",
                                     tag=f"av{i}", bufs=1)
                            for i in range(2)
                        ]
                    for fn in inj.get((qcp, kt), ()):
                        fn()
                    if do_D:
                        if kt == 10:
                            state["o_big"] = opool.tile(
                                [128, KT_D, 512], F16, name="o_big",
                                tag="o_big")
                            emit_D_pair(qc - 1, 0)
                        elif kt == 11:
                            emit_D_pair(qc - 1, 2)
                        elif kt == 12:
                            emit_D_pair(qc - 1, 4)
                            nc.sync.dma_start(
                                out_re[:, qc - 1, 0:4, :],
                                state["o_big"][:, 0:4, :])
                        elif kt == 13:
                            emit_D_pair(qc - 1, 6)
                        elif kt == 15:
                            nc.sync.dma_start(
                                out_re[:, qc - 1, 4:8, :],
                                state["o_big"][:, 4:8, :])
                    if kt >= LAG:
                        av_mms(cur, kt - LAG)
                prev = cur

            # tail
            for kt in range(KT_S - LAG, KT_S):
                av_mms(prev, kt)
            normalize(prev)
            state["o_big"] = opool.tile(
                [128, KT_D, 512], F16, name="o_big", tag="o_big")
            for ot0 in (0, 2):
                emit_D_pair(QC - 1, ot0)
            nc.sync.dma_start(
                out_re[:, QC - 1, 0:4, :], state["o_big"][:, 0:4, :])
            for ot0 in (4, 6):
                emit_D_pair(QC - 1, ot0)
            nc.scalar.dma_start(
                out_re[:, QC - 1, 4:8, :], state["o_big"][:, 4:8, :])

    nc.compile()
    return nc


_NC = None


def _get_nc():
    global _NC
    if _NC is None:
        _NC = build_nc()
    return _NC


def _swz_w(a, inner):
    """[inner*128, C] -> [128, inner*C] partition-major."""
    rows, C = a.shape
    return np.ascontiguousarray(
        a.reshape(inner, 128, C).transpose(1, 0, 2).reshape(128, inner * C))


def _swz_tok(a):
    """[1024, 2048] -> [512, 4096]: row cc*128+p, col kt*512+c."""
    return np.ascontiguousarray(
        a.reshape(8, 128, 4, 512).transpose(2, 1, 0, 3).reshape(512, 4096))


def kernel(q, k, v, Wq, bq, Wk, bk, Wv, bv, Wo, bo):
    nc = _get_nc()

    q = np.asarray(q, np.float32)
    k = np.asarray(k, np.float32)
    v = np.asarray(v, np.float32)

    xT = {}
    for b in range(B):
        xT[("q", b)] = _swz_tok(np.ascontiguousarray(q[b].T).astype(np.float16))
        xT[("k", b)] = _swz_tok(np.ascontiguousarray(k[b].T).astype(np.float16))
        xT[("v", b)] = _swz_tok(np.ascontiguousarray(v[b].T).astype(np.float16))

    WqT = np.asarray(Wq, np.float32).T.astype(np.float16)
    WkT = np.asarray(Wk, np.float32).T.astype(np.float16)
    WvT = np.asarray(Wv, np.float32).T.astype(np.float16)
    WoT = np.asarray(Wo, np.float32).T.astype(np.float16)
    bq = np.asarray(bq, np.float32)
    bk = np.asarray(bk, np.float32)
    bv = np.asarray(bv, np.float32)
    bo = np.asarray(bo, np.float32)

    in_maps = []
    for c in range(NC_):
        b, g = divmod(c, 4)
        sl = slice(g * HG, (g + 1) * HG)
        bqs, bks = bq[sl], bk[sl]
        bqv_a = np.stack(
            [bqs[0:128], bqs[128:256], bks[0:128], bks[128:256]], axis=1)
        bo_a = (bo if g == 0 else np.zeros_like(bo)).reshape(KT_D, 128).T
        wqk = np.concatenate(
            [_swz_w(np.ascontiguousarray(WqT[:, sl]), KT_D),
             _swz_w(np.ascontiguousarray(WkT[:, sl]), KT_D)], axis=1)
        wvo = np.concatenate(
            [_swz_w(np.ascontiguousarray(WvT[:, sl]), KT_D),
             _swz_w(np.ascontiguousarray(WoT[sl, :]), 2)], axis=1)
        bias_m = np.concatenate(
            [bqv_a, np.broadcast_to(bv[sl], (128, HG)), bo_a],
            axis=1).astype(np.float32)
        in_maps.append({
            "xqT": xT[("q", b)],
            "xkT": xT[("k", b)],
            "xvT": xT[("v", b)],
            "wqkT": np.ascontiguousarray(wqk),
            "wvoT": np.ascontiguousarray(wvo),
            "biasT": np.ascontiguousarray(bias_m),
        })

    res = run_bass_kernel_spmd(nc, in_maps, list(range(NC_)))

    out = np.empty((B, S, D), np.float32)
    for b in range(B):
        acc = np.zeros((128, QC, KT_D, 512), np.float32)
        for g in range(4):
            acc += res.results[b * 4 + g]["outT"].astype(np.float32).reshape(
                128, QC, KT_D, 512)
        oT = acc.transpose(2, 0, 1, 3).reshape(D, S)
        out[b] = oT.T
    return out
